# revision 1
# baseline (speedup 1.0000x reference)
"""Multi-resolution 3D conv (3x3x3, Cin=Cout=16) on 8 TRN2 NeuronCores.

Strategy:
- 8 cores = 4 batches x 2 z-halves (SPMD, identical shapes per core).
- Per level (R in 16/32/48/64): the 3x3x3 conv is decomposed into 9
  PSUM-accumulated matmuls, one per (kz, ky) tap pair. The kx (3-tap)
  conv is folded into a block-Toeplitz stationary matrix:
    K = 8 input x-positions x 16 Cin  = 128 partitions
    M = 6 output x-positions x 16 Cout = 96
  so each streamed column does 128x96 MACs. z/y shifts are free-dim AP
  offsets into one shared SBUF tile (no data replication, no masking).
- All halo/boundary padding is prepared host-side (zero pads + cross-half
  z halos sliced from the full input).
"""

import os
import sys

for _p in ("/opt/trn_rl_repo",):
    if os.path.isdir(_p) and _p not in sys.path:
        sys.path.insert(0, _p)

import numpy as np

import concourse.bacc as bacc
import concourse.bass as bass
import concourse.mybir as mybir
from concourse.bass_utils import run_bass_kernel_spmd
from concourse.tile import TileContext

RESOLUTIONS = (16, 32, 48, 64)
B, C = 4, 16
N_TOTAL = sum(r**3 for r in RESOLUTIONS)
XBO, XBI = 6, 8  # output / input x-positions per x-block
F32 = mybir.dt.float32

# nz: output z-rows per matmul chunk (nz * R <= 512, the fp32 moving-op max)
NZ = {64: 8, 48: 8, 32: 16, 16: 8}


class _Lvl:
    def __init__(self, R):
        self.R = R
        self.Xp = -(-R // XBO)          # number of x-blocks
        self.Zo = R // 2                # output z-rows per core
        self.Zi = self.Zo + 2           # padded input z
        self.Yi = R + 2                 # padded input y
        self.Xin = XBO * self.Xp + 2    # padded input x


LVLS = {R: _Lvl(R) for R in RESOLUTIONS}
ORDER = (64, 48, 32, 16)  # big level first keeps PE warm


def _build_nc():
    nc = bacc.Bacc(target_bir_lowering=False)
    wt = nc.dram_tensor("wt", [128, 9, 96], F32, kind="ExternalInput")
    bs = nc.dram_tensor("bias", [96, 1], F32, kind="ExternalInput")
    xs, ys = {}, {}
    for R in ORDER:
        lv = LVLS[R]
        xs[R] = nc.dram_tensor(
            f"x{R}", [lv.Xin * C, lv.Zi, lv.Yi], F32, kind="ExternalInput"
        )
        ys[R] = nc.dram_tensor(
            f"y{R}", [lv.Xp * 96, lv.Zo, lv.R], F32, kind="ExternalOutput"
        )

    with TileContext(nc) as tc:
        with (
            tc.tile_pool(name="wp", bufs=1) as wp,
            tc.tile_pool(name="ip", bufs=6) as ip,
            tc.tile_pool(name="op", bufs=6) as op,
            tc.tile_pool(name="pp", bufs=4, space="PSUM") as pp,
            tc.tile_pool(name="dp", bufs=1, space="PSUM") as dp,
        ):
            wt_sb = wp.tile([128, 9, 96], F32, name="wt_sb")
            nc.sync.dma_start(wt_sb[:, :, :], wt[:, :, :])
            bs_sb = wp.tile([96, 1], F32, name="bs_sb")
            nc.sync.dma_start(bs_sb[:, :], bs[:, :])
            # The S3 LDWEIGHTS slot only fits one semaphore wait, so make
            # sure every real matmul needs at most one: absorb each DMA's
            # completion wait with a tiny throwaway PE/DVE op first.
            dps = dp.tile([1, 2], F32, name="dps")
            scr = wp.tile([96, 1], F32, name="scr")
            nc.tensor.matmul(
                dps[0:1, 0:1], wt_sb[:, 0, 0:1], wt_sb[:, 0, 0:1],
                start=True, stop=True,
            )
            nc.vector.tensor_copy(scr[:, :], bs_sb[:, :])

            for R in ORDER:
                lv = LVLS[R]
                nz = NZ[R]
                for xb in range(lv.Xp):
                    it = ip.tile(
                        [128, lv.Zi, lv.Yi], F32, tag="it", name=f"it{R}_{xb}"
                    )
                    nc.sync.dma_start(
                        it[:, :, :],
                        xs[R][xb * 96 : xb * 96 + 128, :, :],
                    )
                    nc.tensor.matmul(
                        dps[0:1, 0:1], it[:, 0, 0:1], it[:, 0, 0:1],
                        start=True, stop=True,
                    )
                    for zc in range(0, lv.Zo, nz):
                        ps = pp.tile([96, nz, R], F32, tag="ps", name=f"ps{R}_{xb}_{zc}")
                        for t in range(9):
                            a, b = t // 3, t % 3
                            nc.tensor.matmul(
                                ps[:, :, :],
                                wt_sb[:, t, :],
                                it[:, zc + a : zc + a + nz, b : b + R],
                                start=(t == 0),
                                stop=(t == 8),
                            )
                        ot = op.tile([96, nz, R], F32, tag="ot", name=f"ot{R}_{xb}_{zc}")
                        nc.vector.tensor_scalar_add(ot[:, :, :], ps[:, :, :], bs_sb[:, :])
                        nc.sync.dma_start(
                            ys[R][xb * 96 : (xb + 1) * 96, zc : zc + nz, :],
                            ot[:, :, :],
                        )
    nc.finalize()
    return nc


_NC = None


def _get_nc():
    global _NC
    if _NC is None:
        _NC = _build_nc()
    return _NC


def _build_wt(weight):
    # weight [Cout, Cin, kz, ky, kx]; WT[xi*16+ci, t, xo*16+co] = w[co,ci,a,b,xi-xo]
    w = np.asarray(weight, np.float32)
    WT = np.zeros((XBI, C, 9, XBO, C), np.float32)
    for t in range(9):
        a, b = t // 3, t % 3
        for xo in range(XBO):
            for d in range(3):
                WT[xo + d, :, t, xo, :] = w[:, :, a, b, d].T
    return np.ascontiguousarray(WT.reshape(128, 9, 96))


def _prep_in_maps(inp, weight, bias):
    WT = _build_wt(weight)
    b96 = np.ascontiguousarray(
        np.tile(np.asarray(bias, np.float32), XBO).reshape(96, 1)
    )
    # per-level dense grids [B, R, R, R, C]
    grids = {}
    off = 0
    for R in RESOLUTIONS:
        n = R**3
        grids[R] = inp[:, off : off + n, :].reshape(B, R, R, R, C)
        off += n

    in_maps = []
    for core in range(8):
        bi, h = core // 2, core % 2
        m = {"wt": WT, "bias": b96}
        for R in RESOLUTIONS:
            lv = LVLS[R]
            x = grids[R][bi]
            P = np.zeros((lv.Zi, lv.Yi, lv.Xin, C), np.float32)
            zlo = h * lv.Zo - 1
            s0, s1 = max(zlo, 0), min(zlo + lv.Zi, R)
            P[s0 - zlo : s1 - zlo, 1 : R + 1, 1 : R + 1, :] = x[s0:s1]
            m[f"x{R}"] = np.ascontiguousarray(
                P.reshape(lv.Zi, lv.Yi, lv.Xin * C).transpose(2, 0, 1)
            )
        in_maps.append(m)
    return in_maps


def _gather(results):
    out = np.empty((B, N_TOTAL, C), np.float32)
    for core, res in enumerate(results):
        bi, h = core // 2, core % 2
        off = 0
        for R in RESOLUTIONS:
            lv = LVLS[R]
            y = res[f"y{R}"].reshape(lv.Xp * XBO, C, lv.Zo, R)
            y = y.transpose(2, 3, 0, 1)[:, :, :R, :]  # [Zo, y, x, C]
            rows = lv.Zo * R * R
            out[bi, off + h * rows : off + (h + 1) * rows, :] = y.reshape(rows, C)
            off += R**3
    return out


_EXEC = None


def _get_exec():
    """Build the jitted 8-core shard_map executable once and cache it.

    Mirrors concourse.bass2jax.run_bass_via_pjrt but keeps the jitted
    function alive so repeat kernel() calls skip retrace/recompile.
    """
    global _EXEC
    if _EXEC is not None:
        return _EXEC
    import jax
    from jax.sharding import Mesh, PartitionSpec
    from jax.experimental.shard_map import shard_map
    from concourse import bass2jax as b2j
    import concourse.mybir as _mybir

    nc = _get_nc()
    b2j.install_neuronx_cc_hook()

    part_name = nc.partition_id_tensor.name if nc.partition_id_tensor else None
    in_names, out_names, out_avals, zero_shapes = [], [], [], []
    for alloc in nc.m.functions[0].allocations:
        if not isinstance(alloc, _mybir.MemoryLocationSet):
            continue
        name = alloc.memorylocations[0].name
        if alloc.kind == "ExternalInput":
            if name != part_name:
                in_names.append(name)
        elif alloc.kind == "ExternalOutput":
            out_names.append(name)
            shape = tuple(alloc.tensor_shape)
            dtype = _mybir.dt.np(alloc.dtype)
            out_avals.append(jax.core.ShapedArray(shape, dtype))
            zero_shapes.append((shape, dtype))
    n_params = len(in_names)
    n_outs = len(out_names)
    all_names = in_names + out_names
    if part_name is not None:
        all_names = all_names + [part_name]
    donate = tuple(range(n_params, n_params + n_outs))

    def _body(*args):
        operands = list(args)
        if part_name is not None:
            operands.append(b2j.partition_id_tensor())
        outs = b2j._bass_exec_p.bind(
            *operands,
            out_avals=tuple(out_avals),
            in_names=tuple(all_names),
            out_names=tuple(out_names),
            lowering_input_output_aliases=(),
            sim_require_finite=True,
            sim_require_nnan=True,
            nc=nc,
        )
        return tuple(outs)

    devices = jax.devices()[:8]
    mesh = Mesh(np.asarray(devices), ("core",))
    specs = (PartitionSpec("core"),) * (n_params + n_outs)
    sharded = jax.jit(
        shard_map(
            _body, mesh=mesh, in_specs=specs,
            out_specs=(PartitionSpec("core"),) * n_outs,
            check_rep=False,
        ),
        donate_argnums=donate,
        keep_unused=True,
    )
    _EXEC = (sharded, in_names, out_names, out_avals, zero_shapes)
    return _EXEC


def _execute(in_maps):
    sharded, in_names, out_names, out_avals, zero_shapes = _get_exec()
    concat_in = [
        np.concatenate([in_maps[c][n] for c in range(8)], axis=0) for n in in_names
    ]
    concat_zeros = [
        np.zeros((8 * s[0], *s[1:]), d) for s, d in zero_shapes
    ]
    out_arrs = sharded(*concat_in, *concat_zeros)
    return [
        {
            name: np.asarray(out_arrs[i]).reshape(8, *out_avals[i].shape)[c]
            for i, name in enumerate(out_names)
        }
        for c in range(8)
    ]


def _run(inputs, trace=False):
    inp = np.asarray(inputs["input"], np.float32)
    weight = np.asarray(inputs["weight"], np.float32)
    bias = np.asarray(inputs["bias"], np.float32)
    in_maps = _prep_in_maps(inp, weight, bias)
    results = _execute(in_maps)
    return _gather(results), None


def kernel(**inputs):
    out, _ = _run(inputs)
    return out



# revision 3
# speedup vs baseline: 2.3759x; 2.3759x over previous
"""Multi-resolution 3D conv (3x3x3, Cin=Cout=16) on 8 TRN2 NeuronCores.

Strategy:
- 8 cores = 4 batches x 2 z-halves. Each core runs an INDEPENDENT
  single-device bass exec (no shard_map barrier), so host->device upload,
  compute, and device->host download of different cores pipeline over the
  (slow, full-duplex) axon tunnel.
- All tunnel traffic is fp16 (tolerance 2e-2 >> fp16 error ~5e-4):
  one fused input tensor per core [weights | bias | 4 level slabs], one
  fused output tensor per core. No zero output buffers are uploaded:
  every output element is written by the kernel, so persistent on-device
  dummies stand in for the zero-init operands.
- Per level (R in 16/32/48/64): 3x3x3 conv = 9 PSUM-accumulated matmuls,
  one per (kz, ky) tap. The kx 3-tap conv is folded into a block-Toeplitz
  stationary matrix: K = 8 input x-positions x 16 Cin = 128 partitions,
  M = 6 output x-positions x 16 Cout = 96. The last x-block is SHIFTED to
  x = R-6 (overlapping the previous block) so the padded x extent is
  exactly R+2 for every level.
"""

import os
import sys
import threading
from concurrent.futures import ThreadPoolExecutor

for _p in ("/opt/trn_rl_repo",):
    if os.path.isdir(_p) and _p not in sys.path:
        sys.path.insert(0, _p)

import numpy as np

import concourse.bacc as bacc
import concourse.mybir as mybir
from concourse.tile import TileContext

RESOLUTIONS = (16, 32, 48, 64)
B, C = 4, 16
N_TOTAL = sum(r**3 for r in RESOLUTIONS)
XBO, XBI = 6, 8  # output / input x-positions per x-block
F16 = mybir.dt.float16
F32 = mybir.dt.float32

# nz: output z-rows per matmul chunk (nz * R <= 512, one PSUM bank)
NZ = {64: 8, 48: 8, 32: 16, 16: 8}

WT_ELEMS = 128 * 9 * 96
BS_ELEMS = 96


class _Lvl:
    def __init__(self, R):
        self.R = R
        self.Xp = -(-R // XBO)          # number of x-blocks
        self.Zo = R // 2                # output z-rows per core
        self.Zi = self.Zo + 2           # z rows incl halo
        self.Yi = R + 2                 # padded y
        self.Xin = R + 2                # padded x (last block shifted)
        self.in_elems = self.Xin * C * self.Zi * self.Yi
        self.out_elems = self.Xp * 96 * self.Zo * self.R

    def xo(self, xb):
        return min(xb * XBO, self.R - XBO)


LVLS = {R: _Lvl(R) for R in RESOLUTIONS}
ORDER = (64, 48, 32, 16)  # big level first keeps PE warm

# fused input layout per core: [wt | bias | lvl64 | lvl48 | lvl32 | lvl16]
XOFF = {}
_o = WT_ELEMS + BS_ELEMS
for _R in ORDER:
    XOFF[_R] = _o
    _o += LVLS[_R].in_elems
XN = _o
YOFF = {}
_o = 0
for _R in ORDER:
    YOFF[_R] = _o
    _o += LVLS[_R].out_elems
YN = _o

LOFF = {}  # level offset in the flat [B, N_TOTAL, C] input
_o = 0
for _R in RESOLUTIONS:
    LOFF[_R] = _o
    _o += _R**3


def _build_nc():
    nc = bacc.Bacc(target_bir_lowering=False)
    xin = nc.dram_tensor("xin", [XN], F16, kind="ExternalInput")
    yout = nc.dram_tensor("yout", [YN], F16, kind="ExternalOutput")

    wt = xin[0:WT_ELEMS].rearrange("(k t m) -> k t m", t=9, m=96)
    bs = xin[WT_ELEMS : WT_ELEMS + BS_ELEMS].rearrange("(p o) -> p o", o=1)
    xs, ys = {}, {}
    for R in ORDER:
        lv = LVLS[R]
        xs[R] = xin[XOFF[R] : XOFF[R] + lv.in_elems].rearrange(
            "(p z y) -> p z y", z=lv.Zi, y=lv.Yi
        )
        ys[R] = yout[YOFF[R] : YOFF[R] + lv.out_elems].rearrange(
            "(p z y) -> p z y", z=lv.Zo, y=lv.R
        )

    with TileContext(nc) as tc:
        with (
            tc.tile_pool(name="wp", bufs=1) as wp,
            tc.tile_pool(name="ip", bufs=6) as ip,
            tc.tile_pool(name="op", bufs=6) as op,
            tc.tile_pool(name="pp", bufs=4, space="PSUM") as pp,
            tc.tile_pool(name="dp", bufs=1, space="PSUM") as dp,
        ):
            wt_sb = wp.tile([128, 9, 96], F16, name="wt_sb")
            nc.sync.dma_start(wt_sb[:, :, :], wt)
            bs_sb = wp.tile([96, 1], F16, name="bs_sb")
            nc.sync.dma_start(bs_sb[:, :], bs)
            # The S3 LDWEIGHTS slot only fits one semaphore wait, so make
            # sure every real matmul needs at most one: absorb each DMA's
            # completion wait with a tiny throwaway PE/DVE op first.
            dps = dp.tile([1, 2], F32, name="dps")
            scr = wp.tile([96, 1], F32, name="scr")
            nc.tensor.matmul(
                dps[0:1, 0:1], wt_sb[:, 0, 0:1], wt_sb[:, 0, 0:1],
                start=True, stop=True,
            )
            # scr doubles as the fp32 bias used by every tensor_scalar_add
            nc.vector.tensor_copy(scr[:, :], bs_sb[:, :])

            for R in ORDER:
                lv = LVLS[R]
                nz = NZ[R]
                for xb in range(lv.Xp):
                    xo = lv.xo(xb)
                    it = ip.tile(
                        [128, lv.Zi, lv.Yi], F16, tag="it", name=f"it{R}_{xb}"
                    )
                    nc.sync.dma_start(
                        it[:, :, :],
                        xs[R][xo * C : xo * C + 128, :, :],
                    )
                    nc.tensor.matmul(
                        dps[0:1, 0:1], it[:, 0, 0:1], it[:, 0, 0:1],
                        start=True, stop=True,
                    )
                    for zc in range(0, lv.Zo, nz):
                        ps = pp.tile([96, nz, R], F32, tag="ps", name=f"ps{R}_{xb}_{zc}")
                        for t in range(9):
                            a, b = t // 3, t % 3
                            nc.tensor.matmul(
                                ps[:, :, :],
                                wt_sb[:, t, :],
                                it[:, zc + a : zc + a + nz, b : b + R],
                                start=(t == 0),
                                stop=(t == 8),
                            )
                        ot = op.tile([96, nz, R], F16, tag="ot", name=f"ot{R}_{xb}_{zc}")
                        nc.vector.tensor_scalar_add(ot[:, :, :], ps[:, :, :], scr[:, :])
                        nc.sync.dma_start(
                            ys[R][xb * 96 : (xb + 1) * 96, zc : zc + nz, :],
                            ot[:, :, :],
                        )
    nc.finalize()
    return nc


def _build_wt(weight):
    # weight [Cout, Cin, kz, ky, kx]; WT[xi*16+ci, t, xo*16+co] = w[co,ci,a,b,xi-xo]
    w = np.asarray(weight, np.float32)
    WT = np.zeros((XBI, C, 9, XBO, C), np.float16)
    for t in range(9):
        a, b = t // 3, t % 3
        for xo_ in range(XBO):
            for d in range(3):
                WT[xo_ + d, :, t, xo_, :] = w[:, :, a, b, d].T
    return np.ascontiguousarray(WT.reshape(WT_ELEMS))


_ST = None  # lazy global state
_ST_LOCK = threading.Lock()


class _State:
    def __init__(self):
        import jax
        from concourse import bass2jax as b2j

        self.jax = jax
        nc = _build_nc()
        b2j.install_neuronx_cc_hook()
        # the kernel body never reads the partition id; bind it as 0
        part_name = (
            nc.partition_id_tensor.name if nc.partition_id_tensor is not None else None
        )
        in_names = ("xin", "yout") + ((part_name,) if part_name else ())

        out_aval = jax.core.ShapedArray((YN,), np.float16)

        def _body(xin_arr, ydummy):
            operands = [xin_arr, ydummy]
            if part_name is not None:
                operands.append(b2j.partition_id_tensor())
            outs = b2j._bass_exec_p.bind(
                *operands,
                out_avals=(out_aval,),
                in_names=in_names,
                out_names=("yout",),
                lowering_input_output_aliases=(),
                sim_require_finite=True,
                sim_require_nnan=True,
                nc=nc,
            )
            return outs[0]

        self.jfn = jax.jit(_body, keep_unused=True)
        self.devs = jax.devices()[:8]

        # persistent on-device stand-ins for the zero-init output operand
        # (every output element is DMA-written by the kernel, so their
        # contents never reach the result)
        mk = jax.jit(lambda a: jax.numpy.broadcast_to(a, (YN,)))
        self.dummies = []
        for d in self.devs:
            anchor = jax.device_put(np.zeros((), np.float16), d)
            self.dummies.append(jax.block_until_ready(mk(anchor)))

        # host buffers: fused per-core input rows + per-core-level scratch
        self.XG = np.zeros((8, XN), np.float16)
        self.P = {}
        for core in range(8):
            for R in RESOLUTIONS:
                lv = LVLS[R]
                self.P[(core, R)] = np.zeros(
                    (lv.Zi, lv.Yi, lv.Xin, C), np.float16
                )
        self.fetch_pool = ThreadPoolExecutor(8)
        self.exec_pool = ThreadPoolExecutor(8)


def _get_state():
    global _ST
    if _ST is None:
        with _ST_LOCK:
            if _ST is None:
                _ST = _State()
    return _ST


def _pack_core(st, core, inp):
    bi, h = core // 2, core % 2
    row = st.XG[core]
    for R in RESOLUTIONS:
        lv = LVLS[R]
        x = inp[bi, LOFF[R] : LOFF[R] + R**3].reshape(R, R, R, C)
        P = st.P[(core, R)]
        zlo = h * lv.Zo - 1
        s0, s1 = max(zlo, 0), min(zlo + lv.Zi, R)
        P[s0 - zlo : s1 - zlo, 1 : R + 1, 1 : R + 1, :] = x[s0:s1]
        dst = row[XOFF[R] : XOFF[R] + lv.in_elems].reshape(
            lv.Xin * C, lv.Zi, lv.Yi
        )
        np.copyto(dst, P.reshape(lv.Zi, lv.Yi, lv.Xin * C).transpose(2, 0, 1))


def _unpack_core(core, ya, out):
    bi, h = core // 2, core % 2
    for R in RESOLUTIONS:
        lv = LVLS[R]
        rows = lv.Zo * R * R
        dst = out[
            bi, LOFF[R] + h * rows : LOFF[R] + (h + 1) * rows
        ].reshape(lv.Zo, R, R, C)
        src = ya[YOFF[R] : YOFF[R] + lv.out_elems].reshape(
            lv.Xp, XBO, C, lv.Zo, R
        )
        for xb in range(lv.Xp - 1):
            np.copyto(
                dst[:, :, xb * XBO : (xb + 1) * XBO, :],
                src[xb].transpose(2, 3, 0, 1),
            )
        np.copyto(dst[:, :, R - XBO :, :], src[lv.Xp - 1].transpose(2, 3, 0, 1))


def _run(inputs, trace=False):
    st = _get_state()
    jax = st.jax
    inp = np.asarray(inputs["input"], np.float32)
    weight = np.asarray(inputs["weight"], np.float32)
    bias = np.asarray(inputs["bias"], np.float32)

    wt_row = _build_wt(weight)
    b96 = np.tile(bias.astype(np.float16), XBO)
    for core in range(8):
        st.XG[core, 0:WT_ELEMS] = wt_row
        st.XG[core, WT_ELEMS : WT_ELEMS + BS_ELEMS] = b96

    out = np.empty((B, N_TOTAL, C), np.float32)

    def _exec_and_fetch(core, xdev):
        ydev = st.jfn(xdev, st.dummies[core])
        ya = np.asarray(ydev)
        _unpack_core(core, ya, out)

    futs = []
    for core in range(8):
        _pack_core(st, core, inp)
        xdev = jax.device_put(st.XG[core], st.devs[core])
        futs.append(st.exec_pool.submit(_exec_and_fetch, core, xdev))
    for f in futs:
        f.result()
    return out, None


def kernel(**inputs):
    out, _ = _run(inputs)
    return out
